# revision 1
# baseline (speedup 1.0000x reference)
"""CrossAttention Trainium2 kernel (8 NeuronCores, SPMD).

Reference computation (per batch b):
    kv   = code[b].T @ W1          -> k, v        [S, H, HD] each
    q    = inputs[b].T @ W2        -> q           [S, H, HD]
    attn = softmax(q @ k.T / sqrt(HD))            per head
    out  = (attn @ v) reshaped @ W3               [S, D]
    return out.T                                   [D, S]

Sharding: 8 cores = 2 batches x 4 head-groups (4 heads each).
Each core computes its batch's q/k/v for its 4 heads (W1/W2 column-split),
the attention for those heads, and a partial W3 product (W3 row-split).
The host sums the 4 partials per batch (the "all-reduce") -- this is part
of unsharding the row-split W3 output.  Partials ship as bf16; the host
accumulates in fp32.

On-device layout is feature-major (transposed), matching the [D, S] input
layout, so no transposes are ever needed:
    kT[hd, s], qT[hd, s]  (2 head-pair tiles of 128 partitions)
    scores computed k-major: E[k_chunk, q] = exp(scale * kT.T@qT)
    numerator O[hd(+den), q] = [v|1].T @ E   (ones column gives softmax denom)
    normalize by the denom row, then outT_partial = W3_g.T @ O_stack.

Scheduling: the ScalarE exp stream is the critical resource (~147us busy).
Emission order makes it start early (x/code DMA ordered for head-0's needs,
q/k head-pair-0 projections first, head-0 scores emitted before the rest of
the projections) and keeps it dense (all PSUM->SBUF copies during the
attention phase go to VectorE, not ScalarE).  Output-stack merge DMAs ride
the gpsimd SWDGE queue so they never head-of-line-block the SP queue that
feeds input tiles.
"""

import os
import ml_dtypes
import numpy as np
from contextlib import ExitStack

import concourse.bass as bass
import concourse.bacc as bacc
import concourse.mybir as mybir
import concourse.tile as tile
from concourse.bass_utils import run_bass_kernel_spmd

P = 128
D = 1024
S = 2048
H_TOTAL = 16
HD = 64
B = 2
NCORES = 8
GROUPS = 4                  # head groups (tensor parallel axis)
NH = H_TOTAL // GROUPS      # 4 heads per core
FH = NH * HD                # 256 projection features per core
DC = D // P                 # 8 contraction chunks of the model dim
NKC = S // P                # 16 key chunks
NQS = S // 512              # 4 q slices of 512
SCALE = float(HD) ** -0.5

F32 = mybir.dt.float32
F32R = mybir.dt.float32r
BF16 = mybir.dt.bfloat16

FP8 = mybir.dt.float8e4

EXP = mybir.ActivationFunctionType.Exp
EBIAS = -2.772588722239781  # -ln(16): scales E by 1/16, cancels in softmax


def _emit(tc, nc, code, x, w1k, w1v, w2, w3, out):
    with ExitStack() as ctx:
        # ---------------- persistent tiles ----------------
        keep = ctx.enter_context(tc.tile_pool(name="keep", bufs=1))
        w3_sb = keep.tile([P, 2 * D], BF16, tag="w3", name="w3sb")

        kt_sb = [keep.tile([P, S], F32R, tag=f"kt{p}", name=f"kt{p}") for p in range(2)]
        qt_sb = [keep.tile([P, S], F32R, tag=f"qt{p}", name=f"qt{p}") for p in range(2)]
        # v in sequence-major layout, heads interleaved as [v_h (64) | 1];
        # fp8 pair tiles (key chunks 2i | 2i+1) for DoubleRow AV matmuls.
        v_sb = [keep.tile([P, NH * (HD + 1)], BF16, tag=f"v{i}", name=f"v{i}")
                for i in range(NKC)]
        # normalized O stack, fp8 (scaled 8x for e4m3 range), kc-interleaved
        # [kc0 | kc1] in one tile so the fp8-DoubleRow W3 matmul can pair them
        ostack = keep.tile([P, 2 * S], BF16, tag="os", name="ostack")

        # ---------------- input DMA + early projections ----------------
        xpool = ctx.enter_context(tc.tile_pool(name="xin", bufs=4))
        inp = ctx.enter_context(tc.tile_pool(name="inblk", bufs=4))
        wq_sb = keep.tile([P, DC * FH], BF16, tag="w2", name="w2sb")
        wp = ctx.enter_context(tc.tile_pool(name="wproj", bufs=1))
        w1k_sb = wp.tile([P, DC * FH], BF16, tag="w1k")
        w1v_sb = wp.tile([P, DC * FH], BF16, tag="w1v")
        w2_sb = wq_sb
        def dma_w(wsb_t, wdram):
            nc.sync.dma_start(
                wsb_t[:].rearrange("p (d f) -> p d f", f=FH),
                wdram.rearrange("(d p) f -> p d f", p=P))
        dma_w(w2_sb, w2)

        # DMA order tuned for the head-0 exp stream: first scores chunk needs
        # code block 0 (keys 0-511) and x blocks 0-1 (queries 0-1023).
        xblks, cblks = [None] * 4, [None] * 4
        def dma_x(b):
            xblk_t = xpool.tile([P, DC * 512], BF16, tag="xblk", name=f"xblk{b}",
                                bufs=4)
            xblks[b] = [xblk_t[:, d * 512:(d + 1) * 512] for d in range(DC)]
            nc.sync.dma_start(
                xblk_t[:].rearrange("p (d s) -> p d s", s=512),
                x.rearrange("(d p) s -> p d s", p=P)[:, :, b * 512:(b + 1) * 512])
        def dma_code(b):
            blk_t = inp.tile([P, DC * 512], BF16, tag="blk", name=f"cblk{b}")
            cblks[b] = [blk_t[:, d * 512:(d + 1) * 512] for d in range(DC)]
            nc.sync.dma_start(
                blk_t[:].rearrange("p (d s) -> p d s", s=512),
                code.rearrange("(d p) s -> p d s", p=P)[:, :, b * 512:(b + 1) * 512])

        dma_x(0); dma_w(w1k_sb, w1k); dma_code(0); dma_x(1); dma_x(2); dma_x(3)
        dma_code(1); dma_code(2); dma_code(3)
        dma_w(w1v_sb, w1v)

        # constant bias column for the fp8-ranged exp
        ebias_sb = keep.tile([P, 1], F32, tag="ebias", name="ebias")
        nc.vector.memset(ebias_sb[:], EBIAS)

        # scores PSUM pool opens FIRST so its 4 banks never wait on the
        # projection pools' releases (bank reuse would gate the exp stream).
        scp_cm = tc.tile_pool(name="sc", bufs=2, space="PSUM")
        scp = scp_cm.__enter__()

        warm_sb = keep.tile([P, 512], BF16, tag="warm", name="warm")
        nc.vector.memset(warm_sb[:], 0.0)

        # ALL projections up front, in background priority; per-block order
        # q-pair0/k-pair0 first so the head-0 exp stream starts earliest.
        # The scheduler slots these into PE gaps under the exp stream.
        with tc.tile_pool(name="pproj", bufs=4, space="PSUM") as pp:
            wps = pp.tile([P, 512], F32, tag="pk", name="warmps")
            for i in range(16):
                nc.tensor.matmul(wps[:], warm_sb[:, 0:P], warm_sb[:],
                                 start=(i == 0), stop=(i == 15))
            for b in range(4):
                ps = pp.tile([P, 512], F32, tag="pk", name=f"pq{b}_0")
                for d in range(DC):
                    nc.tensor.matmul(
                        ps[:], (w2_sb[:, d * FH: d * FH + P]), (xblks[b][d]),
                        start=(d == 0), stop=(d == DC - 1))
                with tc.high_priority():
                    nc.vector.tensor_copy(qt_sb[0][:, b * 512:(b + 1) * 512], ps[:])
                ps = pp.tile([P, 512], F32, tag="pk", name=f"pk{b}_0")
                for d in range(DC):
                    nc.tensor.matmul(
                        ps[:], (w1k_sb[:, d * FH: d * FH + P]), (cblks[b][d]),
                        start=(d == 0), stop=(d == DC - 1))
                with tc.high_priority():
                    nc.vector.tensor_copy(kt_sb[0][:, b * 512:(b + 1) * 512], ps[:])
            for b in range(4):
                ps = pp.tile([P, 512], F32, tag="pk", name=f"pk{b}_1")
                for d in range(DC):
                    nc.tensor.matmul(
                        ps[:], (w1k_sb[:, d * FH + P: d * FH + 2 * P]),
                        (cblks[b][d]), start=(d == 0), stop=(d == DC - 1))
                nc.vector.tensor_copy(kt_sb[1][:, b * 512:(b + 1) * 512], ps[:])
                ps = pp.tile([P, 512], F32, tag="pk", name=f"pq{b}_1")
                for d in range(DC):
                    nc.tensor.matmul(
                        ps[:], (wq_sb[:, d * FH + P: d * FH + 2 * P]),
                        (xblks[b][d]), start=(d == 0), stop=(d == DC - 1))
                nc.vector.tensor_copy(qt_sb[1][:, b * 512:(b + 1) * 512], ps[:])
                for j in range(4):
                    st = b * 4 + j
                    ps = pp.tile([P, FH], F32, tag="pk", name=f"pv{st}")
                    with tc.high_priority(1 << 19):
                        for d in range(DC):
                            nc.tensor.matmul(
                                ps[:], (cblks[b][d][:, j * P:(j + 1) * P]),
                                (w1v_sb[:, d * FH:(d + 1) * FH]),
                                start=(d == 0), stop=(d == DC - 1))
                    dst = v_sb[st][:].rearrange("p (h c) -> p h c", c=HD + 1)
                    nc.vector.tensor_copy(dst[:, :, 0:HD],
                                          ps[:].rearrange("p (h c) -> p h c", c=HD))
                    nc.vector.memset(dst[:, :, HD:HD + 1], 1.0)

        # W3 weights load late -- only needed at the output projection.
        nc.sync.dma_start(
            w3_sb[:].rearrange("p (j f) -> p j f", f=D),
            w3.rearrange("(j p) f -> p j f", p=P))

        # ---------------- attention ----------------
        e_tiles = {}
        o_psum = {}
        HIPRI = 1 << 20

        def emit_scores_chunk(h, c, epool, scp):
            p, r0 = h // 2, (h % 2) * 64
            if c % 4 == 0:
                e_tiles[(h, c // 4)] = epool.tile([P, 4 * S], BF16, tag="e",
                                                  name=f"e{h}_{c // 4}")
            et = e_tiles[(h, c // 4)]
            # two half-chunks, double-buffered PSUM: the next half's matmuls
            # overlap this half's exp, keeping ScalarE saturated.
            with tc.high_priority(HIPRI):
                for hf in range(2):
                    sc = scp.tile([P, S // 2], F32, tag="sc", name=f"sc{h}_{c}_{hf}")
                    for q2 in range(2):
                        qs = hf * 2 + q2
                        nc.tensor.matmul(
                            sc[:, q2 * 512:(q2 + 1) * 512],
                            (kt_sb[p][r0:r0 + 64, c * P:(c + 1) * P]),
                            (qt_sb[p][r0:r0 + 64, qs * 512:(qs + 1) * 512]),
                            start=True, stop=True)
                    nc.scalar.activation(
                        et[:, (c % 4) * S + hf * (S // 2):(c % 4) * S + (hf + 1) * (S // 2)],
                        sc[:], EXP, scale=SCALE)

        def emit_av_chunk(h, c, ovp):
            if c == 0:
                o_psum[h] = ovp.tile([P, S], F32, tag="ov", name=f"o{h}")
            o = o_psum[h]
            et = e_tiles[(h, c // 4)]
            with tc.high_priority(HIPRI):
                for qs in range(NQS):
                    nc.tensor.matmul(
                        o[0:HD + 1, qs * 512:(qs + 1) * 512],
                        v_sb[c][:, h * (HD + 1):(h + 1) * (HD + 1)],
                        et[:, (c % 4) * S + qs * 512:(c % 4) * S + (qs + 1) * 512],
                        start=(c == 0), stop=(c == NKC - 1))

        def emit_norm(h, npool, o4pool):
            o = o_psum.pop(h)
            with tc.high_priority(HIPRI):
                rc = npool.tile([1, S], BF16, tag="recip", name=f"rc{h}")
                with nc.allow_low_precision(reason="softmax denom reciprocal; rel tol 2e-2"):
                    # 8x folds the ostack fp8 range scaling into the denom;
                    # the host divides the final partials by 16*8.
                    nc.vector.reciprocal(rc[:], o[HD:HD + 1, :])
                bc = npool.tile([HD, S], BF16, tag="bc", name=f"bc{h}")
                nc.gpsimd.partition_broadcast(bc[:], rc[:], channels=HD)
                o4 = o4pool.tile([HD, S], BF16, tag="o4", name=f"o4_{h}")
                nc.vector.tensor_mul(o4[:], o[0:HD, :], bc[:])
                # merge into the stacked layout (partition shift); gpsimd
                # SWDGE queue keeps this off the SP input queue.
                nc.gpsimd.dma_start(
                    ostack[(h % 2) * 64:(h % 2) * 64 + 64,
                           (h // 2) * S:(h // 2 + 1) * S], o4[:])

        with tc.tile_pool(name="e", bufs=4) as epool, \
                tc.tile_pool(name="o4", bufs=1) as o4pool, \
                tc.tile_pool(name="norm", bufs=1) as npool:
            with tc.tile_pool(name="ov", bufs=1, space="PSUM") as ovp:
                for c in range(NKC):
                    emit_scores_chunk(0, c, epool, scp)
                for h in range(1, NH):
                    for c in range(NKC):
                        emit_scores_chunk(h, c, epool, scp)
                        emit_av_chunk(h - 1, c, ovp)
                    emit_norm(h - 1, npool, o4pool)
                for c in range(NKC):
                    emit_av_chunk(NH - 1, c, ovp)
                emit_norm(NH - 1, npool, o4pool)
        scp_cm.__exit__(None, None, None)

        # ---------------- output projection (partial W3) ----------------
        # fp8 DoubleRow: each matmul contracts both kc halves (K=256); PSUM
        # accumulates per m-row, then a gpsimd SWDGE store casts f32->bf16
        # straight from PSUM to DRAM (no SBUF staging, no copy engines).
        out3 = out.rearrange("(m p) s -> p m s", p=P)
        with tc.tile_pool(name="w3ps", bufs=2, space="PSUM") as wp3, \
                tc.tile_pool(name="w3st", bufs=2) as wst:
            for m in range(8):
                ps = wp3.tile([P, S], F32, tag="w3ps", name=f"w3ps{m}")
                for qs in range(NQS):
                    for j in range(2):
                        nc.tensor.matmul(
                            ps[:, qs * 512:(qs + 1) * 512],
                            w3_sb[:, j * D + m * P: j * D + (m + 1) * P],
                            ostack[:, j * S + qs * 512: j * S + (qs + 1) * 512],
                            start=(j == 0), stop=(j == 1))
                st = wst.tile([P, S], BF16, tag="w3st", name=f"w3st{m}")
                # tail phase: both copy engines are otherwise idle
                nc.scalar.copy(st[:, 0:1024], ps[:, 0:1024])
                nc.vector.tensor_copy(st[:, 1024:2048], ps[:, 1024:2048])
                nc.sync.dma_start(out3[:, m, :], st[:])


_NC_CACHE = None


def build_nc():
    global _NC_CACHE
    if _NC_CACHE is not None:
        return _NC_CACHE
    nc = bacc.Bacc("TRN2", target_bir_lowering=False, debug=False)
    code = nc.dram_tensor("code", [D, S], BF16, kind="ExternalInput").ap()
    x = nc.dram_tensor("x", [D, S], BF16, kind="ExternalInput").ap()
    w1k = nc.dram_tensor("w1k", [D, FH], BF16, kind="ExternalInput").ap()
    w1v = nc.dram_tensor("w1v", [D, FH], BF16, kind="ExternalInput").ap()
    w2 = nc.dram_tensor("w2", [D, FH], BF16, kind="ExternalInput").ap()
    w3 = nc.dram_tensor("w3", [FH, D], BF16, kind="ExternalInput").ap()
    out = nc.dram_tensor("out", [D, S], BF16, kind="ExternalOutput").ap()
    with tile.TileContext(nc) as tc:
        _emit(tc, nc, code, x, w1k, w1v, w2, w3, out)
    nc.compile()
    _NC_CACHE = nc
    return nc


def _shard_inputs(code, inputs, W1, W2, W3):
    """Build the 8 per-core input maps: core c = batch c//4, head-group c%4."""
    in_maps = []
    for c in range(NCORES):
        b, g = c // GROUPS, c % GROUPS
        cols = slice(g * FH, (g + 1) * FH)
        in_maps.append({
            "code": np.ascontiguousarray(code[b]).astype(ml_dtypes.bfloat16),
            "x": np.ascontiguousarray(inputs[b]).astype(ml_dtypes.bfloat16),
            "w1k": np.ascontiguousarray(W1[:, cols]).astype(ml_dtypes.bfloat16),
            "w1v": np.ascontiguousarray(W1[:, D + g * FH: D + (g + 1) * FH]).astype(ml_dtypes.bfloat16),
            "w2": np.ascontiguousarray(W2[:, cols]).astype(ml_dtypes.bfloat16),
            "w3": np.ascontiguousarray(W3[g * FH:(g + 1) * FH, :]).astype(ml_dtypes.bfloat16),
        })
    return in_maps


def run(code, inputs, W1, W2, W3, trace=False):
    """Returns (full output [B, D, S] fp32, BassKernelResults)."""
    nc = build_nc()
    in_maps = _shard_inputs(code, inputs, W1, W2, W3)
    res = run_bass_kernel_spmd(nc, in_maps, core_ids=list(range(NCORES)), trace=trace)
    parts = [res.results[c]["out"].astype(np.float32) for c in range(NCORES)]
    full = np.stack([
        parts[b * GROUPS] + parts[b * GROUPS + 1] + parts[b * GROUPS + 2] + parts[b * GROUPS + 3]
        for b in range(B)
    ]).astype(np.float32)
    return full, res


def kernel(code, inputs, W1, W2, W3):
    full, _ = run(np.asarray(code), np.asarray(inputs), np.asarray(W1),
                  np.asarray(W2), np.asarray(W3), trace=False)
    return full

